# revision 7
# baseline (speedup 1.0000x reference)
"""Trainium2 Bass kernel for nn_CommunicationLayer (gnn_message_passing).

Computes, for A=3 agents over batch B with feature dim D=128:
    total       = sum_a x_a                      # [1, B, D]
    mean_others = (total - x_i) / (A-1)          # [A, B, D]
    out_i       = x_i + mean_others_i @ W + b    # [A, B, D]

Rewritten as   out_i = x_i + sum_{j != i} x_j @ (W/(A-1)) + b
so the whole computation is 3 accumulating matmuls per batch tile plus one
residual add; no total/mean tensors are ever materialized.

Distribution: data-parallel over the batch axis across 8 NeuronCores
(no cross-device communication), weights replicated.

Per-core dataflow (batch-major chunks of 2048 rows, 3 MiB DMAs):
  DMA in  -> PE transpose (fp32, exact) -> ACT copy PSUM->SBUF
          -> 3x f32r matmul (rhs = [W'|W'], N=256 -> 1 cyc/row PE fast path)
             accumulating into strided PSUM column blocks (per-element
             has_written handles the overlap pattern)
          -> DVE tensor_add (residual x_i + PSUM evacuation fused)
          -> DMA out
"""

import numpy as np

import concourse.bacc as bacc
import concourse.bass as bass
import concourse.mybir as mybir
from concourse.tile import TileContext
from concourse.masks import make_identity
from concourse.bass_utils import run_bass_kernel_spmd

A = 3
B = 524288
D = 128
NCORES = 8
BC = B // NCORES          # 65536 batch rows per core
CHUNK = 2048              # batch rows per chunk
W_PER = CHUNK // 128      # 16 rows per partition per chunk
NCHUNK = BC // CHUNK      # 32
NQUAD = W_PER // 4        # 4 quads of 4 groups per chunk

F32 = mybir.dt.float32
F32R = mybir.dt.float32r


def build_bass():
    # Bacc (not plain Bass): its compile pipeline moves matmul waits onto
    # ldweights and splits >1-wait sync conditions into event semaphores,
    # which the fused 4-byte matmuls need to pass walrus codegen.
    nc = bacc.Bacc(None, target_bir_lowering=False)

    x_ext = nc.declare_dram_parameter("x", [A, BC, D], F32, isOutput=False)
    m_ext = nc.declare_dram_parameter("m", [D, 2 * D], F32, isOutput=False)
    y_ext = nc.declare_dram_parameter("y", [A, BC, D], F32, isOutput=True)

    with TileContext(nc) as tc:
        with (
            tc.tile_pool(name="const", bufs=1) as cpool,
            tc.tile_pool(name="xin_pool", bufs=2) as in_pool,
            tc.tile_pool(name="xout_pool", bufs=2) as out_pool,
            tc.tile_pool(name="xt_pool", bufs=6) as xt_pool,
            tc.tile_pool(name="tpsum_pool", bufs=4, space="PSUM") as tpsum_pool,
            tc.tile_pool(name="mpsum_pool", bufs=4, space="PSUM") as mpsum_pool,
        ):
            ident = cpool.tile([128, 128], F32)
            make_identity(nc, ident)

            mw_f = cpool.tile([D, 2 * D], F32)
            nc.sync.dma_start(out=mw_f, in_=m_ext[:, :])
            # Walrus requires f32r matmul operands to be produced as f32r;
            # the ACT copy performs the rounding cast.
            mw_r = cpool.tile([D, 2 * D], F32R)
            nc.scalar.copy(out=mw_r, in_=mw_f)

            for c in range(NCHUNK):
                b0 = c * CHUNK
                xin = in_pool.tile([128, A * CHUNK], F32, tag="xin")
                src = x_ext[:, b0:b0 + CHUNK, :].rearrange(
                    "a (p w) d -> p a (w d)", p=128
                )
                nc.sync.dma_start(
                    out=xin.rearrange("p (a f) -> p a f", a=A), in_=src
                )

                xout = out_pool.tile([128, A * CHUNK], F32, tag="xout")
                xin4 = xin.rearrange("p (a w d) -> p a w d", a=A, d=D)
                xout4 = xout.rearrange("p (a w d) -> p a w d", a=A, d=D)

                for q in range(NQUAD):
                    # Transpose 4 groups x 3 agents into feature-major tiles.
                    xts = []
                    for j in range(A):
                        tp = tpsum_pool.tile([128, 512], F32, tag="tp")
                        for g4 in range(4):
                            g = q * 4 + g4
                            nc.tensor.transpose(
                                tp[:, g4 * 128:(g4 + 1) * 128],
                                xin[:, j * CHUNK + g * 128:
                                    j * CHUNK + (g + 1) * 128],
                                ident,
                            )
                        xt = xt_pool.tile([128, 512], F32R, tag="xt")
                        nc.scalar.copy(out=xt, in_=tp)
                        xts.append(xt)

                    for g4 in range(4):
                        g = q * 4 + g4
                        ps = mpsum_pool.tile([128, A * D], F32, tag="ps")
                        ps_r = ps.rearrange("p (i d) -> p i d", d=D)
                        # agent j contributes x_j @ W' to output blocks i != j
                        mm_outs = [
                            ps_r[:, 1:3, :],    # j=0 -> blocks 1,2
                            ps_r[:, 0::2, :],   # j=1 -> blocks 0,2
                            ps_r[:, 0:2, :],    # j=2 -> blocks 0,1
                        ]
                        for j in range(A):
                            nc.tensor.matmul(
                                mm_outs[j],
                                lhsT=xts[j][:, g4 * 128:(g4 + 1) * 128],
                                rhs=mw_r,
                                start=(j == 0),
                                stop=(j == A - 1),
                                skip_group_check=True,
                            )
                        # Fused residual add + PSUM->SBUF evacuation.
                        nc.vector.tensor_add(
                            out=xout4[:, :, g, :],
                            in0=ps_r,
                            in1=xin4[:, :, g, :],
                        )

                dst = y_ext[:, b0:b0 + CHUNK, :].rearrange(
                    "a (p w) d -> p a (w d)", p=128
                )
                nc.sync.dma_start(
                    out=dst, in_=xout.rearrange("p (a f) -> p a f", a=A)
                )

    # Bacc defers register allocation to its compile() pass (run by
    # finalize); the PJRT exec path serializes nc as-is, so finalize here.
    nc.finalize()
    return nc


def run(inputs, trace=False):
    """Build, compile, and run on 8 cores. Returns (full_output, results_obj)."""
    agent_states = np.asarray(inputs["agent_states"], dtype=np.float32)
    W = np.asarray(inputs["W"], dtype=np.float32)
    b = np.asarray(inputs["b"], dtype=np.float32)

    wp = (W * (1.0 / (A - 1))).astype(np.float32)
    m_host = np.ascontiguousarray(np.concatenate([wp, wp], axis=1))

    nc = build_bass()

    in_maps = []
    for i in range(NCORES):
        shard = np.ascontiguousarray(agent_states[:, i * BC:(i + 1) * BC, :])
        in_maps.append({"x": shard, "m": m_host})

    res = run_bass_kernel_spmd(nc, in_maps, list(range(NCORES)), trace=trace)

    out = np.concatenate([r["y"] for r in res.results], axis=1)
    if np.any(b):
        out = out + b.reshape(1, 1, D)
    return out, res


def kernel(**inputs):
    out, _ = run(inputs, trace=False)
    return out
